# revision 1
# baseline (speedup 1.0000x reference)
"""CrossAttention (B=2, N=M=2048, 16 heads x 64) on 8 TRN2 NeuronCores.

Sharding: data-parallel over batch (2) x tensor-parallel over heads (4 per
core). Each core computes q/k/v projections for its 4 heads, streaming
softmax(QK^T)V in a transposed (feature-major) layout, and a partial output
projection against its row-slice of Wo. Partial outputs are summed on host.

Layout trick: all activations are kept feature-major (transposed), so every
matmul has its contraction dim on SBUF partitions and no on-device transpose
is ever needed. x/context are transposed on host; the output is produced
transposed and un-transposed on host.

Softmax: logits are small (|logit| < ~3), so exp() without max-subtraction is
numerically safe. The softmax denominator is obtained for free by augmenting
V with a ones-column (M=65 in the PV matmul): PSUM row 64 accumulates
sum(exp). Normalization happens on the tiny [64, 512] PV output, not on the
[2048, 2048] attention matrix.
"""

import sys

if "/opt/trn_rl_repo" not in sys.path:
    sys.path.insert(0, "/opt/trn_rl_repo")

import ml_dtypes
import numpy as np

import concourse.bass as bass
import concourse.mybir as mybir
import concourse.tile as tile
from concourse import bacc
from concourse.bass_utils import run_bass_kernel_spmd

HEADS = 16
DH = 64
QD = 1024  # query/context feature dim
NN = 2048  # query tokens
MM = 2048  # context tokens
NCORES = 8
HPC = HEADS // (NCORES // 2)  # 4 heads per core
HD = HPC * DH  # 256 inner cols per core

BF = mybir.dt.bfloat16
F32 = mybir.dt.float32

_CACHE = {}


def _build(debug_taps=False, phases="all", repeat=1):
    nc = bacc.Bacc("TRN2", target_bir_lowering=False, debug=False)
    xT = nc.declare_dram_parameter("xT", [QD, NN], BF, isOutput=False)
    cT = nc.declare_dram_parameter("cT", [QD, MM], BF, isOutput=False)
    wq = nc.declare_dram_parameter("wq", [QD, HD], BF, isOutput=False)
    wk = nc.declare_dram_parameter("wk", [QD, HD], BF, isOutput=False)
    wv = nc.declare_dram_parameter("wv", [QD, HD], BF, isOutput=False)
    wo = nc.declare_dram_parameter("wo", [HD, QD], BF, isOutput=False)
    out = nc.declare_dram_parameter("out", [QD, NN], F32, isOutput=True)
    taps = None
    if debug_taps:
        taps = {
            "dq": nc.declare_dram_parameter("dq", [128, 2, NN], BF, isOutput=True),
            "dk": nc.declare_dram_parameter("dk", [128, 2, MM], BF, isOutput=True),
            "dv": nc.declare_dram_parameter("dv", [128, MM // 128, HPC, DH + 1], BF, isOutput=True),
            "dpvs": nc.declare_dram_parameter("dpvs", [128, 2, NN], BF, isOutput=True),
        }

    with tile.TileContext(nc) as tc:
        for _ in range(repeat):
            _emit(tc, xT, cT, wq, wk, wv, wo, out, taps, phases)
    nc.compile()
    return nc


def _emit(tc, xT, cT, wq, wk, wv, wo, out, taps=None, phases="all"):
    nc = tc.nc
    Exp = mybir.ActivationFunctionType.Exp
    mult = mybir.AluOpType.mult
    KT = QD // 128  # 8 contraction tiles for projections
    TT = MM // 128  # 16 context-token tiles
    IB = NN // 512  # 4 query-column blocks

    from contextlib import ExitStack
    ctx = ExitStack()
    persist = ctx.enter_context(tc.tile_pool(name="persist", bufs=1))
    xs = persist.tile([128, KT, NN], BF, tag="xs")
    cs = persist.tile([128, KT, MM], BF, tag="cs")
    wqs = persist.tile([128, KT, HD], BF, tag="wqs")
    wks = persist.tile([128, KT, HD], BF, tag="wks")
    wvs = persist.tile([128, KT, HD], BF, tag="wvs")
    wos = persist.tile([128, 2, QD], BF, tag="wos")
    qs = persist.tile([128, 2, NN], BF, tag="qs")  # qT: [head-pair, tokens]
    ks = persist.tile([128, 2, MM], BF, tag="ks")
    vs = persist.tile([128, TT, HPC, DH + 1], BF, tag="vs")  # v + ones col
    pvs = persist.tile([128, 2, NN], BF, tag="pvs")  # normalized attnV^T

    qkp = ctx.enter_context(tc.tile_pool(name="qk_ps", bufs=2, space="PSUM"))
    pvp = ctx.enter_context(tc.tile_pool(name="pv_ps", bufs=2, space="PSUM"))
    projp = ctx.enter_context(tc.tile_pool(name="proj_ps", bufs=2, space="PSUM"))
    expp = ctx.enter_context(tc.tile_pool(name="expp", bufs=37))
    outp = ctx.enter_context(tc.tile_pool(name="outp", bufs=2))
    nrm = ctx.enter_context(tc.tile_pool(name="nrm", bufs=4))

    # ---- loads: per-k interleave of exactly what the first q/k chains
    # consume (wq/wk/x/context), so the chains' k-th accumulation step can
    # fire as each tile lands; v/output weights trail (their consumers are
    # filler units that run well into the first attention call) ----
    # first column-halves gate the upfront q0/k0 chains; second halves feed
    # deferred filler chains and can trail
    H = NN // 2
    for k in range(KT):
        nc.sync.dma_start(wqs[:, k, :], wq[k * 128:(k + 1) * 128, :])
        nc.sync.dma_start(wks[:, k, :], wk[k * 128:(k + 1) * 128, :])
        nc.sync.dma_start(xs[:, k, 0:H], xT[k * 128:(k + 1) * 128, 0:H])
        nc.sync.dma_start(cs[:, k, 0:H], cT[k * 128:(k + 1) * 128, 0:H])
    for k in range(KT):
        nc.sync.dma_start(wvs[:, k, :], wv[k * 128:(k + 1) * 128, :])
    for k in range(KT):
        nc.sync.dma_start(cs[:, k, H:NN], cT[k * 128:(k + 1) * 128, H:NN])
    for k in range(KT):
        nc.sync.dma_start(xs[:, k, H:NN], xT[k * 128:(k + 1) * 128, H:NN])
    for t in range(2):
        nc.sync.dma_start(wos[:, t, :], wo[t * 128:(t + 1) * 128, :])
    nc.gpsimd.memset(vs[:, :, :, DH:DH + 1], 1.0)

    do = lambda p: phases == "all" or p in phases
    if not do("proj"):
        for t in (qs, ks, pvs):
            nc.gpsimd.memset(t[:, :, :], 0.25)
        nc.gpsimd.memset(vs[:, :, :, 0:DH], 0.25)

    def qk_chain(jb, i4, dst, w, src):
        ps = projp.tile([128, 512], F32, tag="proj", name="ps")
        for k in range(KT):
            nc.tensor.matmul(
                ps[:, :],
                lhsT=w[:, k, jb * 128:(jb + 1) * 128],
                rhs=src[:, k, i4 * 512:(i4 + 1) * 512],
                start=(k == 0),
                stop=(k == KT - 1),
            )
        nc.vector.tensor_copy(dst[:, jb, i4 * 512:(i4 + 1) * 512], ps[:, :])

    def qk_proj(jb):
        # q/k projections for head-pair jb (feature-major out: [j, tokens])
        for dst, w, src in ((qs, wqs, xs), (ks, wks, cs)):
            for i4 in range(4):
                qk_chain(jb, i4, dst, w, src)

    def v_chain(tt):
        # v projection for one token tile (token-major out: [tokens, hd])
        ps = projp.tile([128, HPC, DH], F32, tag="proj", name="ps")
        for k in range(KT):
            nc.tensor.matmul(
                ps[:, :, :],
                lhsT=cs[:, k, tt * 128:(tt + 1) * 128],
                rhs=wvs[:, k, :],
                start=(k == 0),
                stop=(k == KT - 1),
            )
        nc.vector.tensor_copy(vs[:, tt, :, 0:DH], ps[:, :, :])

    def v_proj():
        for tt in range(TT):
            v_chain(tt)

    def final_proj(ib):
        for ob in range(QD // 128):
            fp = projp.tile([128, 512], F32, tag="proj", name="fp")
            for t2 in range(2):
                nc.tensor.matmul(
                    fp[:, :],
                    lhsT=wos[:, t2, ob * 128:(ob + 1) * 128],
                    rhs=pvs[:, t2, ib * 512:(ib + 1) * 512],
                    start=(t2 == 0), stop=(t2 == 1),
                )
            ot = outp.tile([128, 512], F32, tag="ot", name="ot")
            nc.vector.tensor_copy(ot[:, :], fp[:, :])
            nc.sync.dma_start(out[ob * 128:(ob + 1) * 128, ib * 512:(ib + 1) * 512], ot[:, :])

    def attn(hp, ib2, first=False, fillers=()):
        # QK^T + exp for all 16 token tiles (2-head row-packed, K=64).
        # `fillers` are deferred work units (projection chains, the previous
        # attention call's PV/norm units, output-projection blocks) emitted
        # evenly across the token loop so the PE retires them while ACT
        # streams this call's exp pass.
        fillers = list(fillers)
        nfill = len(fillers)
        done = 0
        es = {}
        for tt in range(TT):
            qk0 = qkp.tile([128, 1024], F32, tag="qk", name="qk0")
            qk1 = qkp.tile([128, 1024], F32, tag="qk", name="qk1")
            for i01 in range(2):
                c0 = ib2 * 1024 + i01 * 512
                nc.tensor.matmul(
                    qk0[:, i01 * 512:(i01 + 1) * 512],
                    lhsT=ks[0:64, hp, tt * 128:(tt + 1) * 128],
                    rhs=qs[0:64, hp, c0:c0 + 512],
                    start=True, stop=True,
                    tile_position=(0, 0),
                )
                nc.tensor.matmul(
                    qk1[:, i01 * 512:(i01 + 1) * 512],
                    lhsT=ks[64:128, hp, tt * 128:(tt + 1) * 128],
                    rhs=qs[64:128, hp, c0:c0 + 512],
                    start=True, stop=True,
                    tile_position=(64, 0),
                )
            e0 = expp.tile([128, 1024], BF, tag="exp", name="e0")
            nc.scalar.activation(e0[:, :], qk0[:, :], Exp, scale=0.125)
            e1 = expp.tile([128, 1024], BF, tag="exp", name="e1")
            nc.scalar.activation(e1[:, :], qk1[:, :], Exp, scale=0.125)
            es[(tt, 0)], es[(tt, 1)] = e0, e1
            while done < (nfill * (tt + 1)) // TT:
                fillers.pop(0)()
                done += 1
        while fillers:
            fillers.pop(0)()
        return es

    def pv_units(hp, ib2, es):
        # PV + rowsum (M=65 augmented V) + normalize, as small work units
        # suitable for interleaving into the next attn call's exp pass.
        units = []
        for h01 in range(2):
            cell = []

            def mm_unit(tt, h01=h01, cell=cell):
                if tt == 0:
                    cell.append([pvp.tile([DH + 1, 512], F32, tag="pv", name="pv")
                                 for _ in range(2)])
                for i01 in range(2):
                    nc.tensor.matmul(
                        cell[0][i01][:, :],
                        lhsT=vs[:, tt, 2 * hp + h01, :],
                        rhs=es[(tt, h01)][:, i01 * 512:(i01 + 1) * 512],
                        start=(tt == 0), stop=(tt == TT - 1),
                    )

            def norm_unit(h01=h01, cell=cell):
                for i01 in range(2):
                    p = cell[0][i01]
                    c0 = ib2 * 1024 + i01 * 512
                    rc = nrm.tile([1, 512], F32, tag="rc", name="rc")
                    nc.vector.reciprocal(rc[:, :], p[64:65, :])
                    rep = nrm.tile([64, 512], F32, tag="rep", name="rep")
                    nc.gpsimd.partition_broadcast(rep[:, :], rc[:, :])
                    nc.vector.tensor_tensor(
                        pvs[h01 * 64:(h01 + 1) * 64, hp, c0:c0 + 512],
                        p[0:64, :],
                        rep[:, :],
                        mult,
                    )

            units.extend([(lambda tt=tt, f=mm_unit: f(tt)) for tt in range(TT)])
            units.append(norm_unit)
        return units

    def final_units(ib):
        # output projection for one 512-wide query-column block, per ob
        units = []
        for ob in range(QD // 128):
            def u(ob=ob, ib=ib):
                fp = projp.tile([128, 512], F32, tag="proj", name="fp")
                for t2 in range(2):
                    nc.tensor.matmul(
                        fp[:, :],
                        lhsT=wos[:, t2, ob * 128:(ob + 1) * 128],
                        rhs=pvs[:, t2, ib * 512:(ib + 1) * 512],
                        start=(t2 == 0), stop=(t2 == 1),
                    )
                ot = outp.tile([128, 512], F32, tag="ot", name="ot")
                nc.vector.tensor_copy(ot[:, :], fp[:, :])
                nc.sync.dma_start(out[ob * 128:(ob + 1) * 128, ib * 512:(ib + 1) * 512], ot[:, :])
            units.append(u)
        return units

    if do("proj") and do("attn"):
        # minimal prologue: only the q/k chains attn(0,0) reads immediately;
        # the rest ride as fillers so ACT starts as early as possible
        for i4 in range(2):
            qk_chain(0, i4, qs, wqs, xs)
            qk_chain(0, i4, ks, wks, cs)
    elif do("proj"):
        qk_proj(0)
    if do("attn"):
        if do("proj"):
            vfill = [
                (lambda i=i: qk_chain(0, i, ks, wks, cs)) for i in range(2, 4)
            ] + [
                (lambda i=i: qk_chain(0, i, qs, wqs, xs)) for i in range(2, 4)
            ] + [(lambda t=t: v_chain(t)) for t in range(TT)]
            qkfill = [
                (lambda i=i, d=d, w=w, s=s: qk_chain(1, i, d, w, s))
                for d, w, s in ((qs, wqs, xs), (ks, wks, cs))
                for i in range(4)
            ]
        else:
            vfill, qkfill = [], []
        es = attn(0, 0, fillers=vfill)
        es = attn(0, 1, fillers=pv_units(0, 0, es) + qkfill[:2] + qkfill[4:])
        es = attn(1, 0, fillers=pv_units(0, 1, es) + qkfill[2:4])
        es = attn(1, 1, fillers=pv_units(1, 0, es) + (final_units(0) + final_units(1) if do("final") else []))
        for u in pv_units(1, 1, es):
            u()
        if do("final"):
            for u in final_units(2) + final_units(3):
                u()
    elif do("final"):
        for ib in range(IB):
            final_proj(ib)
    if do("proj") and not do("attn"):
        qk_proj(1)
        v_proj()
    if taps is not None:
        nc.sync.dma_start(taps["dq"][:, :, :], qs[:, :, :])
        nc.sync.dma_start(taps["dk"][:, :, :], ks[:, :, :])
        nc.sync.dma_start(taps["dv"][:, :, :, :], vs[:, :, :, :])
        nc.sync.dma_start(taps["dpvs"][:, :, :], pvs[:, :, :])
    ctx.close()


def _inputs_for_core(c, x, context, Wq, Wk, Wv, Wo):
    bf = ml_dtypes.bfloat16
    b, g = c // (NCORES // 2), c % (NCORES // 2)
    sl = slice(g * HD, (g + 1) * HD)
    key = ("xc", b)
    if key not in _CACHE:
        _CACHE[key] = (
            np.ascontiguousarray(x[b].T).astype(bf),
            np.ascontiguousarray(context[b].T).astype(bf),
        )
    xTb, cTb = _CACHE[key]
    return {
        "xT": xTb,
        "cT": cTb,
        "wq": np.ascontiguousarray(Wq[:, sl]).astype(bf),
        "wk": np.ascontiguousarray(Wk[:, sl]).astype(bf),
        "wv": np.ascontiguousarray(Wv[:, sl]).astype(bf),
        "wo": np.ascontiguousarray(Wo[sl, :]).astype(bf),
    }


def kernel(x, context, Wq, Wk, Wv, Wo, bo):
    x = np.asarray(x, np.float32)
    context = np.asarray(context, np.float32)
    if "nc" not in _CACHE:
        _CACHE["nc"] = _build()
    _CACHE.pop(("xc", 0), None)
    _CACHE.pop(("xc", 1), None)
    nc = _CACHE["nc"]
    in_maps = [
        _inputs_for_core(c, x, context, np.asarray(Wq), np.asarray(Wk),
                         np.asarray(Wv), np.asarray(Wo))
        for c in range(NCORES)
    ]
    res = run_bass_kernel_spmd(nc, in_maps, list(range(NCORES))).results
    B = x.shape[0]
    G = NCORES // B
    outp = np.empty((B, NN, QD), np.float32)
    for b in range(B):
        acc = res[b * G]["out"].astype(np.float32)
        for g in range(1, G):
            acc = acc + res[b * G + g]["out"]
        outp[b] = acc.T + np.asarray(bo, np.float32)[None, :]
    return outp



# revision 2
# speedup vs baseline: 1.2068x; 1.2068x over previous
"""CrossAttention (B=2, N=M=2048, 16 heads x 64) on 8 TRN2 NeuronCores.

Sharding: data-parallel over batch (2) x tensor-parallel over heads (4 per
core). Partial outputs (row-slices of Wo) are summed on host.

v2 design, tuned against the TRN2 instruction-cost timeline model:
- QK^T runs in fp8e4(e4m3) DoubleRow mode: the d=64 contraction is folded
  to [32 partitions, 2 halves], which the PE processes at 0.5 cycles/row —
  2x the bf16 rate. Logits are O(1) so e4m3 rounding (~2.4% per element)
  dilutes to ~1% on the final output, within the 2e-2 gate.
- PV is token-major: out[q, d] = P[ktok, q].T @ Vaug[ktok, d+1], so each
  accumulation matmul moves only 65 columns instead of 512 (PE cost is
  proportional to moving columns only; stationary reloads are free in the
  model). The softmax denominator rides along as V's ones-column; the
  normalize is then a per-partition reciprocal+scalar-mul on DVE, and a
  cheap PE transpose brings the result back to feature-major for the
  output projection.
- exp() on ACT is the bottleneck engine (~133us): everything else (PE
  ~115us, DVE ~65us, Pool folds ~52us, DMA ~50us) is scheduled to hide
  under it via an explicit filler plan in the attention units.
- Inputs are host-relayouted to [128, kt, tokens] so each tensor loads
  with O(1) dma_start instructions (SP issue cost is 565ns each).
- A short PE warmup keeps the tensor engine's p-state ramp at full speed
  through the DMA-bound prologue.
"""

import sys

if "/opt/trn_rl_repo" not in sys.path:
    sys.path.insert(0, "/opt/trn_rl_repo")

from contextlib import ExitStack

import ml_dtypes
import numpy as np

import concourse.bass as bass
import concourse.mybir as mybir
import concourse.tile as tile
from concourse import bacc
from concourse.bass_utils import run_bass_kernel_spmd
from concourse.masks import make_identity

HEADS = 16
DH = 64
QD = 1024  # query/context feature dim
NN = 2048  # query tokens
MM = 2048  # context tokens
NCORES = 8
HPC = HEADS // (NCORES // 2)  # 4 heads per core
HD = HPC * DH  # 256 inner cols per core
KT = QD // 128  # 8 contraction tiles for projections
TT = MM // 128  # 16 context-token tiles

BF = mybir.dt.bfloat16
F32 = mybir.dt.float32
F8 = mybir.dt.float8e4

_CACHE = {}


def _build():
    nc = bacc.Bacc("TRN2", target_bir_lowering=False, debug=False)
    xT = nc.declare_dram_parameter("xT", [128, KT, NN], BF, isOutput=False)
    cT = nc.declare_dram_parameter("cT", [128, KT, MM], BF, isOutput=False)
    wq = nc.declare_dram_parameter("wq", [128, KT, HD], BF, isOutput=False)
    wk = nc.declare_dram_parameter("wk", [128, KT, HD], BF, isOutput=False)
    wv = nc.declare_dram_parameter("wv", [128, KT, HD], BF, isOutput=False)
    wo = nc.declare_dram_parameter("wo", [128, 2, QD], BF, isOutput=False)
    out = nc.declare_dram_parameter("out", [QD, NN], F32, isOutput=True)
    with tile.TileContext(nc) as tc:
        _emit(tc, xT, cT, wq, wk, wv, wo, out)
    nc.compile()
    return nc


def _emit(tc, xT, cT, wq, wk, wv, wo, out):
    nc = tc.nc
    Exp = mybir.ActivationFunctionType.Exp
    DR = mybir.MatmulPerfMode.DoubleRow

    ctx = ExitStack()
    persist = ctx.enter_context(tc.tile_pool(name="persist", bufs=1))
    xs = persist.tile([128, KT, NN], BF, tag="xs")
    cs = persist.tile([128, KT, MM], BF, tag="cs")
    wqs = persist.tile([128, KT, HD], BF, tag="wqs")
    wks = persist.tile([128, KT, HD], BF, tag="wks")
    wvs = persist.tile([128, KT, HD], BF, tag="wvs")
    wos = persist.tile([128, 2, QD], BF, tag="wos")
    qs = persist.tile([128, 2, NN], BF, tag="qs")  # [2hd-pair rows, jb, tok]
    ks = persist.tile([128, 2, MM], BF, tag="ks")
    # fp8 folded q/k for DoubleRow QK^T: [jb*32+p, dd, half, tok] where
    # head h = 2*jb+dd lives on partitions jb*32..jb*32+32, contraction
    # element d = half*32 + p.
    qs8 = persist.tile([64, 2, 2, NN], F8, tag="qs8")
    ks8 = persist.tile([64, 2, 2, MM], F8, tag="ks8")
    vs = persist.tile([128, TT, HPC, DH + 1], BF, tag="vs")  # + ones col
    pvs = persist.tile([128, 2, NN], BF, tag="pvs")  # feature-major attnV
    pvn = persist.tile([128, 2, 8, HPC, DH], BF, tag="pvn")  # token-major
    ident = persist.tile([128, 128], BF, tag="ident")
    warm = persist.tile([128, 512], BF, tag="warm")

    qkp = ctx.enter_context(tc.tile_pool(name="qkp", bufs=2, space="PSUM"))
    pvp = ctx.enter_context(tc.tile_pool(name="pvp", bufs=2, space="PSUM"))
    projp = ctx.enter_context(tc.tile_pool(name="projp", bufs=2, space="PSUM"))
    esp = ctx.enter_context(tc.tile_pool(name="esp", bufs=28))
    outp = ctx.enter_context(tc.tile_pool(name="outp", bufs=3))
    nrmp = ctx.enter_context(tc.tile_pool(name="nrmp", bufs=6))

    # ---- DMA issue order = transfer order (single DMA resource).
    # Chosen so the prologue's critical chain (k jb0/i4=0, v0..3, q jb0
    # i4 0..1) is fed as early as possible.
    nc.sync.dma_start(wks[:, :, :], wk[:, :, :])
    nc.sync.dma_start(cs[:, :, 0:512], cT[:, :, 0:512])
    nc.sync.dma_start(wvs[:, :, :], wv[:, :, :])
    nc.sync.dma_start(wqs[:, :, :], wq[:, :, :])
    nc.sync.dma_start(xs[:, :, 0:512], xT[:, :, 0:512])
    nc.sync.dma_start(xs[:, :, 512:1024], xT[:, :, 512:1024])
    nc.sync.dma_start(cs[:, :, 512:1024], cT[:, :, 512:1024])
    nc.sync.dma_start(cs[:, :, 1024:1536], cT[:, :, 1024:1536])
    nc.sync.dma_start(cs[:, :, 1536:2048], cT[:, :, 1536:2048])
    nc.sync.dma_start(xs[:, :, 1024:1536], xT[:, :, 1024:1536])
    nc.sync.dma_start(xs[:, :, 1536:2048], xT[:, :, 1536:2048])
    nc.sync.dma_start(wos[:, :, :], wo[:, :, :])

    nc.gpsimd.memset(warm[:, :], 0.25)
    nc.gpsimd.memset(vs[:, :, :, DH:DH + 1], 1.0)
    make_identity(nc, ident[:, :])

    def warmup():
        wp = projp.tile([128, 512], F32, tag="proj", name="wm")
        nc.tensor.matmul(wp[0:64, :], lhsT=warm[:, 0:64], rhs=warm[:, :],
                         start=True, stop=True)

    def qk_chain(dst, dst8, w, src, jb, i4):
        # q/k projection for head-pair jb, token block i4 (512 wide),
        # staged to bf16 SBUF then folded to fp8 [32, 2, tok] per head.
        ps = projp.tile([128, 512], F32, tag="proj", name="ps")
        for k in range(KT):
            nc.tensor.matmul(
                ps[:, :],
                lhsT=w[:, k, jb * 128:(jb + 1) * 128],
                rhs=src[:, k, i4 * 512:(i4 + 1) * 512],
                start=(k == 0),
                stop=(k == KT - 1),
            )
        c0, c1 = i4 * 512, (i4 + 1) * 512
        nc.vector.tensor_copy(dst[:, jb, c0:c1], ps[:, :])
        for dd in range(2):
            for half in range(2):
                nc.gpsimd.tensor_copy(
                    dst8[jb * 32:(jb + 1) * 32, dd, half, c0:c1],
                    dst[dd * 64 + half * 32:dd * 64 + half * 32 + 32, jb, c0:c1],
                )

    def v_chain(tt):
        ps = projp.tile([128, HPC, DH], F32, tag="proj", name="vp")
        for k in range(KT):
            nc.tensor.matmul(
                ps[:, :, :],
                lhsT=cs[:, k, tt * 128:(tt + 1) * 128],
                rhs=wvs[:, k, :],
                start=(k == 0),
                stop=(k == KT - 1),
            )
        nc.vector.tensor_copy(vs[:, tt, :, 0:DH], ps[:, :, :])

    def attn_unit(ib2, h, fillers=()):
        # QK^T (fp8 DoubleRow) + exp for 16 ktok tiles x 1024 q cols.
        fillers = list(fillers)
        nfill = len(fillers)
        done = 0
        jb, dd = h // 2, h % 2
        es = {}
        for tt in range(TT):
            qk = qkp.tile([128, 1024], F32, tag="qk", name="qk")
            for i01 in range(2):
                c0 = ib2 * 1024 + i01 * 512
                nc.tensor.matmul(
                    qk[:, i01 * 512:(i01 + 1) * 512],
                    lhsT=ks8[jb * 32:(jb + 1) * 32, dd, :, tt * 128:(tt + 1) * 128],
                    rhs=qs8[jb * 32:(jb + 1) * 32, dd, :, c0:c0 + 512],
                    start=True, stop=True,
                    perf_mode=DR,
                )
            e = esp.tile([128, 1024], BF, tag="es", name="es")
            nc.scalar.activation(e[:, :], qk[:, :], Exp, scale=0.125)
            es[tt] = e
            while done < (nfill * (tt + 1)) // TT:
                fillers.pop(0)()
                done += 1
        while fillers:
            fillers.pop(0)()
        return es

    def pv_unit(ib2, h, es, c):
        # token-major PV for one 128-q chunk: accumulate over all ktok
        # tiles, then normalize by the ones-column into pvn.
        pv = pvp.tile([128, DH + 1], F32, tag="pv", name="pv")
        for tt in range(TT):
            nc.tensor.matmul(
                pv[:, :],
                lhsT=es[tt][:, c * 128:(c + 1) * 128],
                rhs=vs[:, tt, h, :],
                start=(tt == 0),
                stop=(tt == TT - 1),
            )
        rc = nrmp.tile([128, 1], F32, tag="rc", name="rc")
        nc.vector.reciprocal(rc[:, :], pv[:, DH:DH + 1])
        nc.vector.tensor_scalar_mul(pvn[:, ib2, c, h, :], pv[:, 0:DH], rc[:, :])

    def tr_unit(ib2, hp, c):
        # transpose one [128 q, 128 (head-pair inner)] tile to feature-major
        tp = projp.tile([128, 128], BF, tag="proj", name="tp")
        nc.tensor.transpose(tp[:, :], pvn[:, ib2, c, 2 * hp:2 * hp + 2, :], ident[:, :])
        nc.vector.tensor_copy(pvs[:, hp, ib2 * 1024 + c * 128:ib2 * 1024 + (c + 1) * 128], tp[:, :])

    def final_unit(ib, ob):
        fp = projp.tile([128, 512], F32, tag="proj", name="fp")
        for t2 in range(2):
            nc.tensor.matmul(
                fp[:, :],
                lhsT=wos[:, t2, ob * 128:(ob + 1) * 128],
                rhs=pvs[:, t2, ib * 512:(ib + 1) * 512],
                start=(t2 == 0), stop=(t2 == 1),
            )
        ot = outp.tile([128, 512], F32, tag="ot", name="ot")
        nc.vector.tensor_copy(ot[:, :], fp[:, :])
        nc.sync.dma_start(out[ob * 128:(ob + 1) * 128, ib * 512:(ib + 1) * 512], ot[:, :])

    # ---- prologue: warmups keep the PE p-state ramp alive through the
    # DMA-bound startup; chains in dependency order.
    for _ in range(10):
        warmup()
    qk_chain(ks, ks8, wks, cs, 0, 0)
    for t in range(4):
        v_chain(t)
    warmup()
    qk_chain(qs, qs8, wqs, xs, 0, 0)
    warmup()
    warmup()
    qk_chain(qs, qs8, wqs, xs, 0, 1)

    F = lambda f, *a: (lambda: f(*a))
    u00 = [F(qk_chain, ks, ks8, wks, cs, 0, 1)] + \
          [F(v_chain, t) for t in (4, 5)] + \
          [F(qk_chain, ks, ks8, wks, cs, 0, 2)] + \
          [F(v_chain, t) for t in (6, 7, 8)] + \
          [F(qk_chain, ks, ks8, wks, cs, 0, 3)] + \
          [F(v_chain, t) for t in (9, 10, 11, 12, 13, 14, 15)]
    es = attn_unit(0, 0, u00)

    u01 = [F(pv_unit, 0, 0, es, c) for c in range(8)] + \
          [F(qk_chain, ks, ks8, wks, cs, 1, 0),
           F(qk_chain, qs, qs8, wqs, xs, 1, 0),
           F(qk_chain, qs, qs8, wqs, xs, 1, 1)]
    es = attn_unit(0, 1, u01)

    u02 = [F(qk_chain, ks, ks8, wks, cs, 1, 1),
           F(pv_unit, 0, 1, es, 0), F(pv_unit, 0, 1, es, 1),
           F(qk_chain, ks, ks8, wks, cs, 1, 2),
           F(pv_unit, 0, 1, es, 2), F(pv_unit, 0, 1, es, 3),
           F(qk_chain, ks, ks8, wks, cs, 1, 3)] + \
          [F(pv_unit, 0, 1, es, c) for c in range(4, 8)]
    es = attn_unit(0, 2, u02)

    u03 = [F(pv_unit, 0, 2, es, c) for c in range(8)] + \
          [F(tr_unit, 0, 0, c) for c in range(8)] + \
          [F(qk_chain, qs, qs8, wqs, xs, 0, 2),
           F(qk_chain, qs, qs8, wqs, xs, 0, 3)]
    es = attn_unit(0, 3, u03)

    u10 = []
    for c in range(8):
        u10.append(F(pv_unit, 0, 3, es, c))
        u10.append(F(tr_unit, 0, 1, c))
    u10 += [F(final_unit, 0, ob) for ob in range(8)]
    u10 += [F(final_unit, 1, ob) for ob in range(8)]
    es = attn_unit(1, 0, u10)

    u11 = [F(pv_unit, 1, 0, es, c) for c in range(8)] + \
          [F(qk_chain, qs, qs8, wqs, xs, 1, 2),
           F(qk_chain, qs, qs8, wqs, xs, 1, 3)]
    es = attn_unit(1, 1, u11)

    u12 = [F(pv_unit, 1, 1, es, c) for c in range(8)]
    es = attn_unit(1, 2, u12)

    u13 = [F(pv_unit, 1, 2, es, c) for c in range(8)] + \
          [F(tr_unit, 1, 0, c) for c in range(8)]
    es = attn_unit(1, 3, u13)

    # ---- epilogue
    for c in range(8):
        pv_unit(1, 3, es, c)
        tr_unit(1, 1, c)
        if c == 3:
            for ob in range(8):
                final_unit(2, ob)
    for ob in range(8):
        final_unit(3, ob)
    ctx.close()


def _relayout(a, kt):
    # [kt*128, F] -> [128, kt, F]
    f = a.shape[1]
    return np.ascontiguousarray(
        a.reshape(kt, 128, f).transpose(1, 0, 2)
    ).astype(ml_dtypes.bfloat16)


def _inputs_for_core(c, x, context, Wq, Wk, Wv, Wo):
    b, g = c // (NCORES // 2), c % (NCORES // 2)
    sl = slice(g * HD, (g + 1) * HD)
    key = ("xc", b)
    if key not in _CACHE:
        _CACHE[key] = (
            _relayout(np.ascontiguousarray(x[b].T), KT),
            _relayout(np.ascontiguousarray(context[b].T), KT),
        )
    xTb, cTb = _CACHE[key]
    return {
        "xT": xTb,
        "cT": cTb,
        "wq": _relayout(np.ascontiguousarray(Wq[:, sl]), KT),
        "wk": _relayout(np.ascontiguousarray(Wk[:, sl]), KT),
        "wv": _relayout(np.ascontiguousarray(Wv[:, sl]), KT),
        "wo": _relayout(np.ascontiguousarray(Wo[sl, :]), 2),
    }


def kernel(x, context, Wq, Wk, Wv, Wo, bo):
    x = np.asarray(x, np.float32)
    context = np.asarray(context, np.float32)
    if "nc" not in _CACHE:
        _CACHE["nc"] = _build()
    _CACHE.pop(("xc", 0), None)
    _CACHE.pop(("xc", 1), None)
    nc = _CACHE["nc"]
    in_maps = [
        _inputs_for_core(c, x, context, np.asarray(Wq), np.asarray(Wk),
                         np.asarray(Wv), np.asarray(Wo))
        for c in range(NCORES)
    ]
    res = run_bass_kernel_spmd(nc, in_maps, list(range(NCORES))).results
    B = x.shape[0]
    G = NCORES // B
    outp = np.empty((B, NN, QD), np.float32)
    for b in range(B):
        acc = res[b * G]["out"].astype(np.float32)
        for g in range(1, G):
            acc = acc + res[b * G + g]["out"]
        outp[b] = acc.T + np.asarray(bo, np.float32)[None, :]
    return outp


# revision 3
# speedup vs baseline: 1.2074x; 1.0005x over previous
"""CrossAttention (B=2, N=M=2048, 16 heads x 64) on 8 TRN2 NeuronCores.

Sharding: data-parallel over batch (2) x tensor-parallel over heads (4 per
core). Partial outputs (row-slices of Wo) are summed on host.

v2 design, tuned against the TRN2 instruction-cost timeline model:
- QK^T runs in fp8e4(e4m3) DoubleRow mode: the d=64 contraction is folded
  to [32 partitions, 2 halves], which the PE processes at 0.5 cycles/row —
  2x the bf16 rate. Logits are O(1) so e4m3 rounding (~2.4% per element)
  dilutes to ~1% on the final output, within the 2e-2 gate.
- PV is token-major: out[q, d] = P[ktok, q].T @ Vaug[ktok, d+1], so each
  accumulation matmul moves only 65 columns instead of 512 (PE cost is
  proportional to moving columns only; stationary reloads are free in the
  model). The softmax denominator rides along as V's ones-column; the
  normalize is then a per-partition reciprocal+scalar-mul on DVE, and a
  cheap PE transpose brings the result back to feature-major for the
  output projection.
- exp() on ACT is the bottleneck engine (~133us): everything else (PE
  ~115us, DVE ~65us, Pool folds ~52us, DMA ~50us) is scheduled to hide
  under it via an explicit filler plan in the attention units.
- Inputs are host-relayouted to [128, kt, tokens] so each tensor loads
  with O(1) dma_start instructions (SP issue cost is 565ns each).
- A short PE warmup keeps the tensor engine's p-state ramp at full speed
  through the DMA-bound prologue.
"""

import sys

if "/opt/trn_rl_repo" not in sys.path:
    sys.path.insert(0, "/opt/trn_rl_repo")

from contextlib import ExitStack

import ml_dtypes
import numpy as np

import concourse.bass as bass
import concourse.mybir as mybir
import concourse.tile as tile
from concourse import bacc
from concourse.bass_utils import run_bass_kernel_spmd
from concourse.masks import make_identity

HEADS = 16
DH = 64
QD = 1024  # query/context feature dim
NN = 2048  # query tokens
MM = 2048  # context tokens
NCORES = 8
HPC = HEADS // (NCORES // 2)  # 4 heads per core
HD = HPC * DH  # 256 inner cols per core
KT = QD // 128  # 8 contraction tiles for projections
TT = MM // 128  # 16 context-token tiles

BF = mybir.dt.bfloat16
F32 = mybir.dt.float32
F8 = mybir.dt.float8e4
F16 = mybir.dt.float16

_CACHE = {}


def _build():
    nc = bacc.Bacc("TRN2", target_bir_lowering=False, debug=False)
    xT = nc.declare_dram_parameter("xT", [128, KT, NN], F16, isOutput=False)
    cT = nc.declare_dram_parameter("cT", [128, KT, MM], F16, isOutput=False)
    wq = nc.declare_dram_parameter("wq", [128, KT, HD], F16, isOutput=False)
    wk = nc.declare_dram_parameter("wk", [128, KT, HD], F16, isOutput=False)
    wv = nc.declare_dram_parameter("wv", [128, KT, HD], F16, isOutput=False)
    wo = nc.declare_dram_parameter("wo", [128, 2, QD], F16, isOutput=False)
    out = nc.declare_dram_parameter("out", [QD, NN], F32, isOutput=True)
    with tile.TileContext(nc) as tc:
        _emit(tc, xT, cT, wq, wk, wv, wo, out)
    nc.compile()
    return nc


def _emit(tc, xT, cT, wq, wk, wv, wo, out):
    nc = tc.nc
    Exp = mybir.ActivationFunctionType.Exp
    DR = mybir.MatmulPerfMode.DoubleRow

    ctx = ExitStack()
    persist = ctx.enter_context(tc.tile_pool(name="persist", bufs=1))
    xs = persist.tile([128, KT, NN], F16, tag="xs")
    cs = persist.tile([128, KT, MM], F16, tag="cs")
    wqs = persist.tile([128, KT, HD], F16, tag="wqs")
    wks = persist.tile([128, KT, HD], F16, tag="wks")
    wvs = persist.tile([128, KT, HD], F16, tag="wvs")
    wos = persist.tile([128, 2, QD], F16, tag="wos")
    qs = persist.tile([128, 2, NN], F16, tag="qs")  # [2hd-pair rows, jb, tok]
    ks = persist.tile([128, 2, MM], F16, tag="ks")
    # fp8 folded q/k for DoubleRow QK^T: [jb*32+p, dd, half, tok] where
    # head h = 2*jb+dd lives on partitions jb*32..jb*32+32, contraction
    # element d = half*32 + p.
    qs8 = persist.tile([32, 2, 2, NN], F8, tag="qs8")
    ks8 = persist.tile([32, 2, 2, MM], F8, tag="ks8")
    vs = persist.tile([128, TT, HPC, DH + 1], F16, tag="vs")  # + ones col
    pvs = persist.tile([128, 2, NN], F16, tag="pvs")  # feature-major attnV
    pvn = persist.tile([128, 2, 8, HPC, DH], F16, tag="pvn")  # token-major
    ident = persist.tile([128, 128], F16, tag="ident")
    warm = persist.tile([128, 512], F16, tag="warm")

    qkp = ctx.enter_context(tc.tile_pool(name="qkp", bufs=2, space="PSUM"))
    pvp = ctx.enter_context(tc.tile_pool(name="pvp", bufs=2, space="PSUM"))
    projp = ctx.enter_context(tc.tile_pool(name="projp", bufs=2, space="PSUM"))
    esp = ctx.enter_context(tc.tile_pool(name="esp", bufs=28))
    outp = ctx.enter_context(tc.tile_pool(name="outp", bufs=3))
    nrmp = ctx.enter_context(tc.tile_pool(name="nrmp", bufs=6))

    # ---- DMA issue order = transfer order (single DMA resource).
    # Chosen so the prologue's critical chain (k jb0/i4=0, v0..3, q jb0
    # i4 0..1) is fed as early as possible.
    nc.sync.dma_start(wks[:, :, :], wk[:, :, :])
    nc.sync.dma_start(cs[:, :, 0:512], cT[:, :, 0:512])
    nc.sync.dma_start(wvs[:, :, :], wv[:, :, :])
    nc.sync.dma_start(wqs[:, :, :], wq[:, :, :])
    nc.sync.dma_start(xs[:, :, 0:512], xT[:, :, 0:512])
    nc.sync.dma_start(xs[:, :, 512:1024], xT[:, :, 512:1024])
    nc.sync.dma_start(cs[:, :, 512:1024], cT[:, :, 512:1024])
    nc.sync.dma_start(cs[:, :, 1024:1536], cT[:, :, 1024:1536])
    nc.sync.dma_start(cs[:, :, 1536:2048], cT[:, :, 1536:2048])
    nc.sync.dma_start(xs[:, :, 1024:1536], xT[:, :, 1024:1536])
    nc.sync.dma_start(xs[:, :, 1536:2048], xT[:, :, 1536:2048])
    nc.sync.dma_start(wos[:, :, :], wo[:, :, :])

    nc.gpsimd.memset(warm[:, :], 0.25)
    nc.gpsimd.memset(vs[:, :, :, DH:DH + 1], 1.0)
    make_identity(nc, ident[:, :])

    def warmup():
        wp = projp.tile([128, 512], F32, tag="proj", name="wm")
        nc.tensor.matmul(wp[0:64, :], lhsT=warm[:, 0:64], rhs=warm[:, :],
                         start=True, stop=True)

    def qk_chain(dst, dst8, w, src, jb, i4):
        # q/k projection for head-pair jb, token block i4 (512 wide),
        # staged to bf16 SBUF then folded to fp8 [32, 2, tok] per head.
        ps = projp.tile([128, 512], F32, tag="proj", name="ps")
        for k in range(KT):
            nc.tensor.matmul(
                ps[:, :],
                lhsT=w[:, k, jb * 128:(jb + 1) * 128],
                rhs=src[:, k, i4 * 512:(i4 + 1) * 512],
                start=(k == 0),
                stop=(k == KT - 1),
            )
        c0, c1 = i4 * 512, (i4 + 1) * 512
        nc.vector.tensor_copy(dst[:, jb, c0:c1], ps[:, :])
        if jb == 0:
            for dd in range(2):
                for half in range(2):
                    nc.gpsimd.tensor_copy(
                        dst8[0:32, dd, half, c0:c1],
                        dst[dd * 64 + half * 32:dd * 64 + half * 32 + 32, jb, c0:c1],
                    )

    def v_chain(tt):
        ps = projp.tile([128, HPC, DH], F32, tag="proj", name="vp")
        for k in range(KT):
            nc.tensor.matmul(
                ps[:, :, :],
                lhsT=cs[:, k, tt * 128:(tt + 1) * 128],
                rhs=wvs[:, k, :],
                start=(k == 0),
                stop=(k == KT - 1),
            )
        nc.vector.tensor_copy(vs[:, tt, :, 0:DH], ps[:, :, :])

    def attn_unit(ib2, h, fillers=()):
        # QK^T (fp8 DoubleRow) + exp for 16 ktok tiles x 1024 q cols.
        fillers = list(fillers)
        nfill = len(fillers)
        done = 0
        jb, dd = h // 2, h % 2
        es = {}
        for tt in range(TT):
            qk = qkp.tile([128, 1024], F32, tag="qk", name="qk")
            for i01 in range(2):
                c0 = ib2 * 1024 + i01 * 512
                if jb == 0:
                    nc.tensor.matmul(
                        qk[:, i01 * 512:(i01 + 1) * 512],
                        lhsT=ks8[0:32, dd, :, tt * 128:(tt + 1) * 128],
                        rhs=qs8[0:32, dd, :, c0:c0 + 512],
                        start=True, stop=True,
                        perf_mode=DR,
                    )
                else:
                    nc.tensor.matmul(
                        qk[:, i01 * 512:(i01 + 1) * 512],
                        lhsT=ks[dd * 64:(dd + 1) * 64, 1, tt * 128:(tt + 1) * 128],
                        rhs=qs[dd * 64:(dd + 1) * 64, 1, c0:c0 + 512],
                        start=True, stop=True,
                    )
            e = esp.tile([128, 1024], F16, tag="es", name="es")
            nc.scalar.activation(e[:, :], qk[:, :], Exp, scale=0.125)
            es[tt] = e
            while done < (nfill * (tt + 1)) // TT:
                fillers.pop(0)()
                done += 1
        while fillers:
            fillers.pop(0)()
        return es

    def pv_unit(ib2, h, es, c):
        # token-major PV for one 128-q chunk: accumulate over all ktok
        # tiles, then normalize by the ones-column into pvn.
        pv = pvp.tile([128, DH + 1], F32, tag="pv", name="pv")
        for tt in range(TT):
            nc.tensor.matmul(
                pv[:, :],
                lhsT=es[tt][:, c * 128:(c + 1) * 128],
                rhs=vs[:, tt, h, :],
                start=(tt == 0),
                stop=(tt == TT - 1),
            )
        rc = nrmp.tile([128, 1], F32, tag="rc", name="rc")
        nc.vector.reciprocal(rc[:, :], pv[:, DH:DH + 1])
        nc.vector.tensor_scalar_mul(pvn[:, ib2, c, h, :], pv[:, 0:DH], rc[:, :])

    def tr_unit(ib2, hp, c):
        # transpose one [128 q, 128 (head-pair inner)] tile to feature-major
        tp = projp.tile([128, 128], F16, tag="proj", name="tp")
        nc.tensor.transpose(tp[:, :], pvn[:, ib2, c, 2 * hp:2 * hp + 2, :], ident[:, :])
        nc.vector.tensor_copy(pvs[:, hp, ib2 * 1024 + c * 128:ib2 * 1024 + (c + 1) * 128], tp[:, :])

    def final_unit(ib, ob):
        fp = projp.tile([128, 512], F32, tag="proj", name="fp")
        for t2 in range(2):
            nc.tensor.matmul(
                fp[:, :],
                lhsT=wos[:, t2, ob * 128:(ob + 1) * 128],
                rhs=pvs[:, t2, ib * 512:(ib + 1) * 512],
                start=(t2 == 0), stop=(t2 == 1),
            )
        ot = outp.tile([128, 512], F32, tag="ot", name="ot")
        nc.vector.tensor_copy(ot[:, :], fp[:, :])
        nc.sync.dma_start(out[ob * 128:(ob + 1) * 128, ib * 512:(ib + 1) * 512], ot[:, :])

    # ---- prologue: warmups keep the PE p-state ramp alive through the
    # DMA-bound startup; chains in dependency order.
    for _ in range(10):
        warmup()
    qk_chain(ks, ks8, wks, cs, 0, 0)
    for t in range(4):
        v_chain(t)
    warmup()
    qk_chain(qs, qs8, wqs, xs, 0, 0)
    warmup()
    warmup()
    qk_chain(qs, qs8, wqs, xs, 0, 1)

    F = lambda f, *a: (lambda: f(*a))
    u00 = [F(qk_chain, ks, ks8, wks, cs, 0, 1)] + \
          [F(v_chain, t) for t in (4, 5)] + \
          [F(qk_chain, ks, ks8, wks, cs, 0, 2)] + \
          [F(v_chain, t) for t in (6, 7, 8)] + \
          [F(qk_chain, ks, ks8, wks, cs, 0, 3)] + \
          [F(v_chain, t) for t in (9, 10, 11, 12, 13, 14, 15)]
    es = attn_unit(0, 0, u00)

    u01 = [F(pv_unit, 0, 0, es, c) for c in range(8)] + \
          [F(qk_chain, ks, ks8, wks, cs, 1, 0),
           F(qk_chain, qs, qs8, wqs, xs, 1, 0),
           F(qk_chain, qs, qs8, wqs, xs, 1, 1)]
    es = attn_unit(0, 1, u01)

    u02 = [F(qk_chain, ks, ks8, wks, cs, 1, 1),
           F(pv_unit, 0, 1, es, 0), F(pv_unit, 0, 1, es, 1),
           F(qk_chain, ks, ks8, wks, cs, 1, 2),
           F(pv_unit, 0, 1, es, 2), F(pv_unit, 0, 1, es, 3),
           F(qk_chain, ks, ks8, wks, cs, 1, 3)] + \
          [F(pv_unit, 0, 1, es, c) for c in range(4, 8)]
    es = attn_unit(0, 2, u02)

    u03 = [F(pv_unit, 0, 2, es, c) for c in range(8)] + \
          [F(tr_unit, 0, 0, c) for c in range(8)] + \
          [F(qk_chain, qs, qs8, wqs, xs, 0, 2),
           F(qk_chain, qs, qs8, wqs, xs, 0, 3)]
    es = attn_unit(0, 3, u03)

    u10 = []
    for c in range(8):
        u10.append(F(pv_unit, 0, 3, es, c))
        u10.append(F(tr_unit, 0, 1, c))
    u10 += [F(final_unit, 0, ob) for ob in range(8)]
    u10 += [F(final_unit, 1, ob) for ob in range(8)]
    es = attn_unit(1, 0, u10)

    u11 = [F(pv_unit, 1, 0, es, c) for c in range(8)] + \
          [F(qk_chain, qs, qs8, wqs, xs, 1, 2),
           F(qk_chain, qs, qs8, wqs, xs, 1, 3)]
    es = attn_unit(1, 1, u11)

    u12 = [F(pv_unit, 1, 1, es, c) for c in range(8)]
    es = attn_unit(1, 2, u12)

    u13 = [F(pv_unit, 1, 2, es, c) for c in range(8)] + \
          [F(tr_unit, 1, 0, c) for c in range(8)]
    es = attn_unit(1, 3, u13)

    # ---- epilogue
    for c in range(8):
        pv_unit(1, 3, es, c)
        tr_unit(1, 1, c)
        if c == 3:
            for ob in range(8):
                final_unit(2, ob)
    for ob in range(8):
        final_unit(3, ob)
    ctx.close()


def _relayout(a, kt):
    # [kt*128, F] -> [128, kt, F]
    f = a.shape[1]
    return np.ascontiguousarray(
        a.reshape(kt, 128, f).transpose(1, 0, 2)
    ).astype(np.float16)


def _inputs_for_core(c, x, context, Wq, Wk, Wv, Wo):
    b, g = c // (NCORES // 2), c % (NCORES // 2)
    sl = slice(g * HD, (g + 1) * HD)
    key = ("xc", b)
    if key not in _CACHE:
        _CACHE[key] = (
            _relayout(np.ascontiguousarray(x[b].T), KT),
            _relayout(np.ascontiguousarray(context[b].T), KT),
        )
    xTb, cTb = _CACHE[key]
    return {
        "xT": xTb,
        "cT": cTb,
        "wq": _relayout(np.ascontiguousarray(Wq[:, sl]), KT),
        "wk": _relayout(np.ascontiguousarray(Wk[:, sl]), KT),
        "wv": _relayout(np.ascontiguousarray(Wv[:, sl]), KT),
        "wo": _relayout(np.ascontiguousarray(Wo[sl, :]), 2),
    }


def kernel(x, context, Wq, Wk, Wv, Wo, bo):
    x = np.asarray(x, np.float32)
    context = np.asarray(context, np.float32)
    if "nc" not in _CACHE:
        _CACHE["nc"] = _build()
    _CACHE.pop(("xc", 0), None)
    _CACHE.pop(("xc", 1), None)
    nc = _CACHE["nc"]
    in_maps = [
        _inputs_for_core(c, x, context, np.asarray(Wq), np.asarray(Wk),
                         np.asarray(Wv), np.asarray(Wo))
        for c in range(NCORES)
    ]
    res = run_bass_kernel_spmd(nc, in_maps, list(range(NCORES))).results
    B = x.shape[0]
    G = NCORES // B
    outp = np.empty((B, NN, QD), np.float32)
    for b in range(B):
        acc = res[b * G]["out"].astype(np.float32)
        for g in range(1, G):
            acc = acc + res[b * G + g]["out"]
        outp[b] = acc.T + np.asarray(bo, np.float32)[None, :]
    return outp
